# revision 65
# baseline (speedup 1.0000x reference)
"""Multi-head attention (b=8, n=1024, d=768, h=12) on 8 Trainium2 NeuronCores.

Strategy: pure data parallelism over the batch — core i computes batch element
i end-to-end (no collectives). Inside each core the computation is laid out
"feature-major" so no on-chip transposes are ever needed:

  - host passes x^T, w_qkv^T, w_proj^T (transposing on host is free input
    marshalling; the PE contracts over the partition dim so both matmul
    operands need the contraction dim partition-major)
  - q^T/k^T are computed feature-major (qkv^T = w_qkv^T.T @ x^T) so S^T tiles
    come straight out of the PE with keys on partitions and queries on the
    free dim; V is computed token-major (x^T.T @ w_v^T) so it is directly the
    PV lhsT. A ones column appended to each head's V makes row 64 of the PV
    accumulator the softmax denominator.
  - softmax is unnormalized exp (logits are O(5) here, exp cannot overflow);
    scale 1/8 is folded into the exp; normalization happens after PV as a
    per-query reciprocal multiply.
  - v_bias and b_proj fold into one effective bias beff = w_proj @ v_bias +
    b_proj added during the projection PSUM->SBUF copy (sum_j softmax = 1).

Perf notes (trace-driven, 206us -> ~176us):
  - S row-group pairing: per (pair, key-tile, query-half) BOTH heads' K=64
    S matmuls write ONE [128,1024] psum tile (h0 cols 0:512, h1 512:1024).
    The two matmuls occupy disjoint PE row groups (tile_position 0/64) AND
    disjoint PSUM banks, so they issue back-to-back and run concurrently
    (HW-measured 110 ns/MM vs 216 serial — the single biggest win, ~20us),
    and one ACT exp instruction covers both heads at no extra cost.
  - phase A streams all four early q/k pairs k-major (pairs 1/7 borrow the
    idle S-psum slots) so each arriving x tile feeds 8 matmuls; x is split
    across the sync/gpsimd DMA queues; the tiny q-bias DMA goes FIRST on the
    scalar queue (behind 2.25MB of weights it stalled the first qk close and
    idled the whole chip ~5us).
  - ~20 warm-up matmuls bridge the ~8us DMA-start latency so the PE's HAM
    clock-gate (4096-cycle activity window) is at 2.4 GHz, not 1.2, when real
    work starts.
  - epilogue: last pair's head-1 PV (whose normalize chain ends in a
    partition-bounce DMA) runs first, and proj k=0..4 partial accumulation
    (needs only attn[0..4]) covers both normalize chains; proj psums
    alternate between the two PSUM pools (4 accumulators in flight); output
    stores alternate between both hwdge DMA queues.
  - a one-instruction Schraudolph exp on DVE (int16(A*x+B) bitcast to bf16,
    ~1.7% rms, softmax-consistent) is available via ATTN_SCHRAUD_JS to offload
    ACT, but measured neutral here (ACT is not the binding engine), so it is
    off by default.
  - CAUTION: some seemingly-innocent schedule variants (e.g. pulling the last
    pair's PV into the main loop, a second warm-fill psum group) flip the
    whole run into a mode where every engine's ops take ~20% longer
    (PE 379->454ns per N=512 matmul).  Measure after every change.
"""

import sys

sys.path.insert(0, "/opt/trn_rl_repo")

import os

import numpy as np
import ml_dtypes

import concourse.bass as bass  # noqa: F401  (import keeps bass registered)
import concourse.mybir as mybir
import concourse.tile as tile
from concourse import bacc
from concourse.bass_utils import run_bass_kernel_spmd

N_CORES = 8
B, N, D = 8, 1024, 768
H, HD = 12, 64
SCALE = HD**-0.5
P = 128
KT = D // P  # 6 contraction tiles over d_model
MQ = 2 * D // P  # 12 output tiles over q+k features
TT = N // P  # 8 token tiles
F32 = mybir.dt.float32
I16 = mybir.dt.int16

# Matmul-operand dtype: "bf16" (bfloat16 operands, fp32 accumulate) or
# "f32r" (fp32 data run through the PE's fast fp32 mode).
DT_MODE = os.environ.get("ATTN_DT", "bf16")
# exp(x) ~= bitcast_bf16(int16(A16*x + B16)): Schraudolph in bf16 bit layout.
SCHRAUD = os.environ.get("ATTN_SCHRAUD", "1") == "1" and DT_MODE == "bf16"
# which key-tiles j run head 1's exp on DVE (Schraudolph); rest go to ACT
SCHRAUD_JS = frozenset(
    int(j) for j in os.environ.get("ATTN_SCHRAUD_JS", "").split(",")
    if j != ""
) if SCHRAUD else frozenset()
A16 = 128.0 / float(np.log(2.0))
# 127<<7 minus the wiggle-centering correction: log2(1+x)-x peaks at 0.0861,
# so c = 0.086/2 * 128 ~= 5.5 centers the piecewise-linear error at ~0 mean
# (a wrong c is a *systematic* scale bias on Schraudolph tiles that softmax
# normalization only cancels if every tile of a head uses the same exp).
B16 = float(os.environ.get("ATTN_B16", str(16256.0 - 5.5)))
WARM = int(os.environ.get("ATTN_WARM", "20"))
# PV-accumulator drain engine: "act" (scalar.copy) or "dve"
DRAIN = os.environ.get("ATTN_DRAIN", "dve")
# dummy filler matmuls per early qk k-step: keep the PE busy (and the HAM
# clock-gate warm) while phase A's input DMAs trickle in
FILL = int(os.environ.get("ATTN_FILL", "0"))
# normalize multiplies on gpsimd instead of DVE
MUL_GPS = os.environ.get("ATTN_MUL_GPS", "0") == "1"


def _np_mm_dtype():
    return ml_dtypes.bfloat16 if DT_MODE == "bf16" else np.float32


def _mm_dt():
    return mybir.dt.bfloat16 if DT_MODE == "bf16" else mybir.dt.float32


def build():
    nc = bacc.Bacc("TRN2", target_bir_lowering=False, debug=False)
    dt = _mm_dt()
    Exp = mybir.ActivationFunctionType.Exp

    def _mm(ap):
        """View an SBUF AP with the dtype actually fed to the tensor engine."""
        if DT_MODE == "f32r":
            return ap.bitcast(mybir.dt.float32r)
        if ap.dtype == I16:
            return ap.bitcast(mybir.dt.bfloat16)
        return ap

    xT_d = nc.dram_tensor("xT", [D, N], dt, kind="ExternalInput")
    wqkT_d = nc.dram_tensor("wqkT", [D, 2 * D], dt, kind="ExternalInput")
    wvT_d = nc.dram_tensor("wvT", [D, D], dt, kind="ExternalInput")
    wpT_d = nc.dram_tensor("wpT", [D, D], dt, kind="ExternalInput")
    qb_d = nc.dram_tensor("qb", [P, KT], F32, kind="ExternalInput")
    beff_d = nc.dram_tensor("beff", [P, D], F32, kind="ExternalInput")
    out_d = nc.dram_tensor("out", [N, D], dt, kind="ExternalOutput")

    with tile.TileContext(nc) as tc:
        with (
            tc.tile_pool(name="psum", bufs=1, space="PSUM") as psum,
            tc.tile_pool(name="persist", bufs=1) as persist,
            tc.tile_pool(name="work", bufs=1) as work,
        ):
            # ---- PE warm-up: dummy matmuls with no DMA dependency ---------------
            wrm_ps = None
            fill_state = {"n": 0, "ps": None}
            if WARM:
                wrm = work.tile([P, 256], dt, tag="wrm", name="wrm")
                nc.vector.memset(wrm[:], 0.0)
                wrm_ps = psum.tile([P, N], F32, tag="mm2", bufs=2, name="wps")

            def emit_fill(n):
                """Dummy matmuls (one long accumulation into the current fill
                psum) that keep the PE streaming while phase-A DMAs land."""
                if wrm_ps is None or fill_state["ps"] is None:
                    return
                for _ in range(n):
                    nc.tensor.matmul(
                        fill_state["ps"][:, 0:256],
                        _mm(wrm)[:, 0:128], _mm(wrm)[:, 0:256],
                        start=(fill_state["n"] == 0), stop=False,
                    )
                    fill_state["n"] += 1

            def fill_finish(drain_cols):
                """Close the open fill group and give its psum a reader (a
                corner of bb, overwritten by the beff DMA afterwards)."""
                if wrm_ps is None or fill_state["ps"] is None:
                    return
                nc.tensor.matmul(
                    fill_state["ps"][:, 0:256],
                    _mm(wrm)[:, 0:128], _mm(wrm)[:, 0:256],
                    start=False, stop=True,
                )
                nc.vector.tensor_copy(bb[:, drain_cols], fill_state["ps"][:, 0:32])
                fill_state["ps"] = None
                fill_state["n"] = 0

            aprobe = None
            if WARM:
                fill_state["ps"] = wrm_ps
                emit_fill(WARM)
                # tiny dummy exp: pulls the ~2.7us ACT table load (+drain)
                # into the DMA-wait window instead of the first real exp
                aprobe = work.tile([1, 4], F32, tag="aprobe", name="aprobe")
                nc.scalar.activation(aprobe[:], wrm[0:1, 0:4], Exp, scale=1.0)

            # ---- input DMAs (ordered by first use across the 3 DGE queues) -----
            # host permutes w_qkv columns pairwise [q0,k0,q1,k1,...] so the
            # first two q/k pairs' weights (wqkA) are contiguous and land
            # first; x streams on sync, weights on scalar, everything not
            # needed until later (wv/wp/beff) rides the slower gpsimd queue.
            # x is the phase-A pace-setter: split it across the sync (x0-2)
            # and gpsimd (x3-5) queues so tiles land in parallel; wv follows
            # x0-2 on sync, wp/beff (needed only at proj) follow x3-5.
            x_sb, wqkA_sb, wqkB_sb, wv_sb = [], [], [], []
            for k in range(KT):
                xk = persist.tile([P, N], dt, tag=f"x{k}", name=f"x{k}")
                x_sb.append(xk)
            # each x tile's halves ride both queues in parallel so a tile
            # lands in ~1.3us instead of ~2.5 (same total bytes per queue)
            for k in range(KT):
                nc.sync.dma_start(
                    x_sb[k][:, 0:512], xT_d.ap()[k * P : (k + 1) * P, 0:512]
                )
                nc.gpsimd.dma_start(
                    x_sb[k][:, 512:N], xT_d.ap()[k * P : (k + 1) * P, 512:N]
                )
            qb_sb = persist.tile([P, KT], F32, tag="qb", name="qb")
            nc.scalar.dma_start(qb_sb[:], qb_d.ap())
            for k in range(KT):
                wqka = persist.tile([P, 512], dt, tag=f"wqkA{k}", name=f"wqkA{k}")
                nc.scalar.dma_start(wqka[:], wqkT_d.ap()[k * P : (k + 1) * P, 0:512])
                wqkA_sb.append(wqka)
            for k in range(KT):
                wvk = persist.tile([P, D], dt, tag=f"wv{k}", name=f"wv{k}")
                nc.sync.dma_start(wvk[:], wvT_d.ap()[k * P : (k + 1) * P, :])
                wv_sb.append(wvk)
            for k in range(KT):
                wqkb = persist.tile([P, 1024], dt, tag=f"wqkB{k}", name=f"wqkB{k}")
                nc.scalar.dma_start(
                    wqkb[:], wqkT_d.ap()[k * P : (k + 1) * P, 512 : 2 * D]
                )
                wqkB_sb.append(wqkb)
            wp_sb = []
            for k in range(KT):
                wpk = work.tile([P, D], dt, tag=f"wp{k}", name=f"wp{k}")
                nc.gpsimd.dma_start(wpk[:], wpT_d.ap()[k * P : (k + 1) * P, :])
                wp_sb.append(wpk)
            bb = work.tile([P, D], F32, tag="bb", name="bb")

            def wqk_slice(m, k):
                """lhsT slice for q/k feature tile m from the pair-permuted
                weight layout: q_m at permuted block 2m, k_m at 2(m-KT)+1."""
                pos = 2 * m if m < KT else 2 * (m - KT) + 1
                if pos < 4:
                    return wqkA_sb[k][:, (pos % 4) * P : (pos % 4 + 1) * P]
                return wqkB_sb[k][:, (pos - 4) * P : (pos - 3) * P]

            # ---- emitters ------------------------------------------------------
            v_sb = [None] * TT

            def emit_v(t):
                vt = persist.tile([P, H, HD + 1], dt, tag=f"v{t}", name=f"v{t}")
                nc.vector.memset(vt[:, :, HD], 1.0)
                ps = psum.tile([P, N], F32, tag="mm2", bufs=2, name=f"vps{t}")
                for k in range(KT):
                    lhsT = _mm(x_sb[k])[:, t * P : (t + 1) * P]
                    nc.tensor.matmul(
                        ps[:, 0:512], lhsT, _mm(wv_sb[k])[:, 0:512],
                        start=(k == 0), stop=(k == KT - 1),
                    )
                    nc.tensor.matmul(
                        ps[:, 512:768], lhsT, _mm(wv_sb[k])[:, 512:768],
                        start=(k == 0), stop=(k == KT - 1),
                    )
                nc.vector.tensor_copy(
                    vt[:, :, 0:HD], ps[:, 0:768].rearrange("p (h d) -> p h d", d=HD)
                )
                v_sb[t] = vt

            qk_sb = [None] * MQ

            def open_qk(m, tag="mm2"):
                """Allocate the psum for q/k tile m; matmuls stream in via
                qk_step so they can be slotted between other work."""
                return psum.tile([P, N], F32, tag=tag, bufs=2, name=f"qkps{m}")

            def qk_step(m, ps, k):
                lhsT = _mm(wqk_slice(m, k))
                for half in range(2):
                    nc.tensor.matmul(
                        ps[:, half * 512 : (half + 1) * 512],
                        lhsT,
                        _mm(x_sb[k])[:, half * 512 : (half + 1) * 512],
                        start=(k == 0), stop=(k == KT - 1),
                    )

            def close_qk(m, ps):
                qkm = persist.tile([P, N], dt, tag=f"qk{m}", name=f"qk{m}")
                if m < KT:
                    # q tile: add q_bias (per-partition scalar in feature-major);
                    # on DVE to keep ACT free for the softmax exps
                    nc.vector.tensor_scalar_add(qkm[:], ps[:], qb_sb[:, m : m + 1])
                else:
                    nc.vector.tensor_copy(qkm[:], ps[:])
                qk_sb[m] = qkm

            def emit_qk(m, fills=0):
                ps = open_qk(m)
                for k in range(KT):
                    qk_step(m, ps, k)
                    emit_fill(fills)
                close_qk(m, ps)

            attn_sb = [
                persist.tile([P, N], dt, tag=f"attn{g}", name=f"attn{g}")
                for g in range(KT)
            ]
            pt_tiles = {}  # (g, j, half) -> exp'd S^T tile, consumed by emit_pv
            pt_reads = {}

            def emit_sx(g, j):
                """S^T matmuls + exp for pair g, key tile j (both heads).

                Layout trick: per query-half, BOTH heads' S go into ONE psum
                tile — h0 at cols 0:512, h1 at cols 512:1024.  The two K=64
                matmuls occupy disjoint PE row groups (tile_position) AND
                disjoint PSUM banks, so they issue back-to-back and run fully
                concurrent (HW-measured 110 ns/MM vs 216 serial).  One exp
                instruction then covers both heads at no extra ACT cost.
                """
                q_t, k_t = qk_sb[g], qk_sb[KT + g]
                for half in range(2):
                    qcols = slice(half * 512, (half + 1) * 512)
                    sp = psum.tile(
                        [P, N], F32, tag="sp", bufs=2, name=f"sp{g}_{j}_{half}"
                    )
                    for hh in range(2):
                        lhsT = _mm(k_t)[hh * HD : (hh + 1) * HD, j * P : (j + 1) * P]
                        rhs = _mm(q_t)[hh * HD : (hh + 1) * HD, qcols]
                        nc.tensor.matmul(
                            sp[:, hh * 512 : (hh + 1) * 512], lhsT, rhs,
                            tile_position=(hh * HD, 0),
                        )
                    if j in SCHRAUD_JS:
                        # Schraudolph exp on DVE: one tensor_scalar; the int16
                        # result IS the bf16 bit pattern of ~exp(x)
                        pt = work.tile(
                            [P, N], I16, tag="pt1", bufs=12, name=f"pt{g}_{j}_{half}"
                        )
                        nc.vector.tensor_scalar(
                            pt[:], sp[:], A16 * SCALE, B16,
                            op0=mybir.AluOpType.mult, op1=mybir.AluOpType.add,
                        )
                    else:
                        pt = work.tile(
                            [P, N], dt, tag="pt", bufs=24, name=f"pt{g}_{j}_{half}"
                        )
                        nc.scalar.activation(pt[:], sp[:], Exp, scale=SCALE)
                    pt_tiles[(g, j, half)] = pt

            def open_pv(g, hh):
                return psum.tile(
                    [HD + 1, N], F32, tag="mm2", bufs=2, name=f"pv{g}_{hh}"
                )

            def pv_step(g, hh, pp, j):
                # pt tiles hold [h0 | h1] per query-half; head hh's probs for
                # query-half q live at ptq[:, hh*512:(hh+1)*512]
                lhsT = _mm(v_sb[j])[:, 2 * g + hh, :]  # [128, 65] V|1
                for half in range(2):
                    # each half-tile is read by both heads; free after 2nd read
                    key = (g, j, half)
                    pt = pt_tiles[key]
                    pt_reads[key] = pt_reads.get(key, 0) + 1
                    if pt_reads[key] == 2:
                        del pt_tiles[key]
                    nc.tensor.matmul(
                        pp[:, half * 512 : (half + 1) * 512],
                        lhsT,
                        _mm(pt)[:, hh * 512 : (hh + 1) * 512],
                        start=(j == 0), stop=(j == TT - 1),
                    )

            def close_pv(g, hh, pp):
                # drain the PSUM accumulator right away so the 2-slot "mm2"
                # rotation unblocks the interleaved q/k matmuls
                sb = work.tile(
                    [HD + 1, N], F32, tag="ppsb", bufs=2, name=f"ppsb{g}{hh}"
                )
                if DRAIN == "act":
                    nc.scalar.copy(sb[:], pp[:])
                else:
                    nc.vector.tensor_copy(sb[:], pp[:])
                # normalize: row HD of sb is the softmax denominator
                rrow0 = work.tile([1, N], F32, tag="rr0", bufs=2, name=f"rr0{g}{hh}")
                rb = work.tile([HD, N], F32, tag="rb", bufs=2, name=f"rb{g}{hh}")
                # s hops to partition 0 (DMA moves across lanes)...
                nc.sync.dma_start(rrow0[:], sb[HD : HD + 1, :])
                # ...1/s at base 0 (approx_fast is ~51 ULP, far below the
                # bf16 noise floor, 5x cheaper than InstReciprocal, and
                # only correct on base-partition-0 APs)...
                nc.vector.reciprocal_approx_fast(rrow0[:], rrow0[:])
                # ...and fan out to all 64 lanes (gpsimd reads partition 0)
                nc.gpsimd.partition_broadcast(rb[:], rrow0[:])
                if hh == 0:
                    nc.vector.tensor_mul(attn_sb[g][0:HD, :], sb[0:HD, :], rb[:])
                else:
                    # PV output lives at partitions 0..64 but this head's
                    # slot in attn_sb is partitions 64..127; DVE lanes
                    # can't shift partitions, so normalize at base 0 and
                    # bounce across partitions with an SBUF->SBUF DMA
                    # (split across both hwdge queues: this bounce sits on
                    # the epilogue critical path before proj's k=5).
                    stg = work.tile([HD, N], dt, tag="stg", bufs=2, name=f"stg{g}")
                    nc.vector.tensor_mul(stg[:], sb[0:HD, :], rb[:])
                    nc.sync.dma_start(attn_sb[g][HD:P, 0:512], stg[:, 0:512])
                    nc.scalar.dma_start(attn_sb[g][HD:P, 512:N], stg[:, 512:N])

            # ---- emission order: software pipeline -----------------------------
            # Tile's per-engine instruction order is (near-)emission order and
            # a stalled instruction head-of-line blocks its engine, so filler
            # work is emitted BEFORE each exp-paced S step. Steady state: pair
            # g's slots carry pair g-1's PV (heads serialized so only one PV
            # accumulator is live -> one free PSUM slot) plus pair g+1's q/k
            # matmuls, so the exp stream never waits on a PE-only block.
            # all four early q/k pairs (they all read the early wqkA block)
            # stream k-major so each x-tile arrival feeds 8 matmuls: pairs 0/6
            # in the mm2 slots, pairs 1/7 borrow the (still unused) sp slots
            fill_finish(slice(0, 32))
            if aprobe is not None:
                # give the probe a reader (bb corner, overwritten by beff DMA)
                nc.vector.tensor_copy(bb[0:1, 60:64], aprobe[:])
            qk_ps = {
                0: open_qk(0), KT: open_qk(KT),
                1: open_qk(1, "sp"), KT + 1: open_qk(KT + 1, "sp"),
            }
            for k in range(KT):
                for m in (0, KT, 1, KT + 1):
                    qk_step(m, qk_ps[m], k)
                emit_fill(FILL)
            for m in (0, KT, 1, KT + 1):
                close_qk(m, qk_ps[m])
            nc.gpsimd.dma_start(bb[:], beff_d.ap())
            for j in range(TT):
                emit_sx(0, j)
                emit_v(j)
            pp51 = None
            for g in range(1, KT):
                nxt = g + 1 if g + 1 < KT else None
                pp = qkps = m = None
                for slot in range(TT):
                    emit_sx(g, slot)
                    hh = 0 if slot < 4 else 1
                    if slot in (0, 4):
                        pp = open_pv(g - 1, hh)
                        if nxt is not None:
                            m = nxt if slot == 0 else KT + nxt
                            qkps = open_qk(m)
                    pv_step(g - 1, hh, pp, 2 * (slot % 4))
                    pv_step(g - 1, hh, pp, 2 * (slot % 4) + 1)
                    if nxt is not None:
                        # finish the k-steps by slot 2/6 so the DVE close cast
                        # lands well before the next iteration's S matmuls
                        # need the q/k tile (this cast was the ~0.9us
                        # iteration-boundary PE stall)
                        for k in ([0, 1, 2], [3, 4], [5], [])[slot % 4]:
                            qk_step(m, qkps, k)
                        if slot in (2, 6):
                            close_qk(m, qkps)
                    if slot in (3, 7):
                        close_pv(g - 1, hh, pp)
            # ---- phase C: out = attn @ w_proj^T + beff -------------------------
            def proj_steps(t, ps, ks):
                for k in ks:
                    lhsT = _mm(attn_sb[k])[:, t * P : (t + 1) * P]
                    nc.tensor.matmul(
                        ps[:, 0:512], lhsT, _mm(wp_sb[k])[:, 0:512],
                        start=(k == 0), stop=(k == KT - 1),
                    )
                    nc.tensor.matmul(
                        ps[:, 512:768], lhsT, _mm(wp_sb[k])[:, 512:768],
                        start=(k == 0), stop=(k == KT - 1),
                    )

            def proj_close(t, ps):
                ot = work.tile([P, D], dt, tag="ot", bufs=3, name=f"ot{t}")
                nc.vector.tensor_add(ot[:], ps[:, 0:768], bb[:])
                # alternate the output stores over the two hwdge queues so
                # the 1.5MB of stores don't serialize into a tail; the very
                # last store splits across both queues to halve the tail
                if t == TT - 1:
                    nc.sync.dma_start(out_d.ap()[t * P : t * P + HD, :], ot[0:HD, :])
                    nc.scalar.dma_start(out_d.ap()[t * P + HD : (t + 1) * P, :], ot[HD:P, :])
                else:
                    eng = (nc.sync, nc.scalar)[t % 2]
                    eng.dma_start(out_d.ap()[t * P : (t + 1) * P, :], ot[:])

            # epilogue: the last pair's PV + normalize interleaved with proj
            # k=0..4 partials (which need only attn[0..4]) so the PE never
            # idles on the exp stream or the normalize chain; proj psums
            # alternate between the two pools -> 4 accumulators in flight.
            def proj_open(t):
                tag = "sp" if t in (0, 1, 4, 6) else "mm2"
                return psum.tile([P, N], F32, tag=tag, bufs=2, name=f"ops{t}")

            # head 1 first: its normalize chain is longer (partition-bounce
            # DMA), so it hides under head 0's PV and the proj partials
            proj_ps = {}
            pp51 = open_pv(KT - 1, 1)
            for j in range(TT):
                pv_step(KT - 1, 1, pp51, j)
            close_pv(KT - 1, 1, pp51)
            pp0 = open_pv(KT - 1, 0)
            for j in range(TT):
                pv_step(KT - 1, 0, pp0, j)
            close_pv(KT - 1, 0, pp0)
            # all k=0..4 partials AFTER both closes: ~8us of ready PE work
            # covering the last normalize chains before anything needs attn[5]
            for t in range(4):
                proj_ps[t] = proj_open(t)
                proj_steps(t, proj_ps[t], range(KT - 1))
            for t in range(4):
                proj_steps(t, proj_ps[t], [KT - 1])
                proj_close(t, proj_ps[t])
            for t in range(4, TT):
                ps = proj_open(t)
                proj_steps(t, ps, range(KT))
                proj_close(t, ps)

    nc.compile()
    return nc


_NC_CACHE = None


def _get_nc():
    global _NC_CACHE
    if _NC_CACHE is None:
        _NC_CACHE = build()
    return _NC_CACHE


def make_in_maps(x, w_qkv, q_bias, v_bias, w_proj, b_proj):
    mmdt = _np_mm_dtype()
    wqkT = np.ascontiguousarray(w_qkv[: 2 * D].T)
    # pair-permute the q/k feature blocks: [q0,k0,q1,k1,...] so the first two
    # pairs' weights are one contiguous early DMA (see wqk_slice in build)
    perm = []
    for m in range(KT):
        perm.extend(range(m * P, (m + 1) * P))
        perm.extend(range((KT + m) * P, (KT + m + 1) * P))
    wqkT = np.ascontiguousarray(wqkT[:, perm]).astype(mmdt)
    wvT = np.ascontiguousarray(w_qkv[2 * D :].T).astype(mmdt)
    wpT = np.ascontiguousarray(w_proj.T).astype(mmdt)
    qb = np.ascontiguousarray(q_bias.reshape(KT, P).T).astype(np.float32)
    beff_row = (
        w_proj.astype(np.float64) @ v_bias.astype(np.float64) + b_proj
    ).astype(np.float32)
    beff = np.ascontiguousarray(np.tile(beff_row, (P, 1)))
    shared = {"wqkT": wqkT, "wvT": wvT, "wpT": wpT, "qb": qb, "beff": beff}
    in_maps = []
    for i in range(N_CORES):
        m = dict(shared)
        m["xT"] = np.ascontiguousarray(x[i].T).astype(mmdt)
        in_maps.append(m)
    return in_maps


def kernel(x, w_qkv, q_bias, v_bias, w_proj, b_proj, _trace=False, _tmpdir=None):
    x = np.asarray(x)
    nc = _get_nc()
    in_maps = make_in_maps(
        np.asarray(x, dtype=np.float32),
        np.asarray(w_qkv, dtype=np.float32),
        np.asarray(q_bias, dtype=np.float32),
        np.asarray(v_bias, dtype=np.float32),
        np.asarray(w_proj, dtype=np.float32),
        np.asarray(b_proj, dtype=np.float32),
    )
    res = run_bass_kernel_spmd(
        nc, in_maps, core_ids=list(range(N_CORES)), trace=_trace, tmpdir=_tmpdir
    )
    out = np.stack(
        [np.asarray(res.results[i]["out"]).astype(np.float32) for i in range(N_CORES)],
        axis=0,
    )
    if _trace:
        return out, res
    return out


# revision 66
# speedup vs baseline: 1.0172x; 1.0172x over previous
"""Multi-head attention (b=8, n=1024, d=768, h=12) on 8 Trainium2 NeuronCores.

Strategy: pure data parallelism over the batch — core i computes batch element
i end-to-end (no collectives). Inside each core the computation is laid out
"feature-major" so no on-chip transposes are ever needed:

  - host passes x^T, w_qkv^T, w_proj^T (transposing on host is free input
    marshalling; the PE contracts over the partition dim so both matmul
    operands need the contraction dim partition-major)
  - q^T/k^T are computed feature-major (qkv^T = w_qkv^T.T @ x^T) so S^T tiles
    come straight out of the PE with keys on partitions and queries on the
    free dim; V is computed token-major (x^T.T @ w_v^T) so it is directly the
    PV lhsT. A ones column appended to each head's V makes row 64 of the PV
    accumulator the softmax denominator.
  - softmax is unnormalized exp (logits are O(5) here, exp cannot overflow);
    scale 1/8 is folded into the exp; normalization happens after PV as a
    per-query reciprocal multiply.
  - v_bias and b_proj fold into one effective bias beff = w_proj @ v_bias +
    b_proj added during the projection PSUM->SBUF copy (sum_j softmax = 1).

Perf notes (trace-driven, 206us -> ~176us):
  - S row-group pairing: per (pair, key-tile, query-half) BOTH heads' K=64
    S matmuls write ONE [128,1024] psum tile (h0 cols 0:512, h1 512:1024).
    The two matmuls occupy disjoint PE row groups (tile_position 0/64) AND
    disjoint PSUM banks, so they issue back-to-back and run concurrently
    (HW-measured 110 ns/MM vs 216 serial — the single biggest win, ~20us),
    and one ACT exp instruction covers both heads at no extra cost.
  - phase A streams all four early q/k pairs k-major (pairs 1/7 borrow the
    idle S-psum slots) so each arriving x tile feeds 8 matmuls; x is split
    across the sync/gpsimd DMA queues; the tiny q-bias DMA goes FIRST on the
    scalar queue (behind 2.25MB of weights it stalled the first qk close and
    idled the whole chip ~5us).
  - ~20 warm-up matmuls bridge the ~8us DMA-start latency so the PE's HAM
    clock-gate (4096-cycle activity window) is at 2.4 GHz, not 1.2, when real
    work starts.
  - epilogue: last pair's head-1 PV (whose normalize chain ends in a
    partition-bounce DMA) runs first, and proj k=0..4 partial accumulation
    (needs only attn[0..4]) covers both normalize chains; proj psums
    alternate between the two PSUM pools (4 accumulators in flight); output
    stores alternate between both hwdge DMA queues.
  - a one-instruction Schraudolph exp on DVE (int16(A*x+B) bitcast to bf16,
    ~1.7% rms, softmax-consistent) is available via ATTN_SCHRAUD_JS to offload
    ACT, but measured neutral here (ACT is not the binding engine), so it is
    off by default.
  - CAUTION: some seemingly-innocent schedule variants (e.g. pulling the last
    pair's PV into the main loop, a second warm-fill psum group) flip the
    whole run into a mode where every engine's ops take ~20% longer
    (PE 379->454ns per N=512 matmul).  Measure after every change.
"""

import sys

sys.path.insert(0, "/opt/trn_rl_repo")

import os

import numpy as np
import ml_dtypes

import concourse.bass as bass  # noqa: F401  (import keeps bass registered)
import concourse.mybir as mybir
import concourse.tile as tile
from concourse import bacc
from concourse.bass_utils import run_bass_kernel_spmd

N_CORES = 8
B, N, D = 8, 1024, 768
H, HD = 12, 64
SCALE = HD**-0.5
P = 128
KT = D // P  # 6 contraction tiles over d_model
MQ = 2 * D // P  # 12 output tiles over q+k features
TT = N // P  # 8 token tiles
F32 = mybir.dt.float32
I16 = mybir.dt.int16

# Matmul-operand dtype: "bf16" (bfloat16 operands, fp32 accumulate) or
# "f32r" (fp32 data run through the PE's fast fp32 mode).
DT_MODE = os.environ.get("ATTN_DT", "bf16")
# exp(x) ~= bitcast_bf16(int16(A16*x + B16)): Schraudolph in bf16 bit layout.
SCHRAUD = os.environ.get("ATTN_SCHRAUD", "1") == "1" and DT_MODE == "bf16"
# which key-tiles j run head 1's exp on DVE (Schraudolph); rest go to ACT
SCHRAUD_JS = frozenset(
    int(j) for j in os.environ.get("ATTN_SCHRAUD_JS", "").split(",")
    if j != ""
) if SCHRAUD else frozenset()
A16 = 128.0 / float(np.log(2.0))
# 127<<7 minus the wiggle-centering correction: log2(1+x)-x peaks at 0.0861,
# so c = 0.086/2 * 128 ~= 5.5 centers the piecewise-linear error at ~0 mean
# (a wrong c is a *systematic* scale bias on Schraudolph tiles that softmax
# normalization only cancels if every tile of a head uses the same exp).
B16 = float(os.environ.get("ATTN_B16", str(16256.0 - 5.5)))
WARM = int(os.environ.get("ATTN_WARM", "20"))
# PV-accumulator drain engine: "act" (scalar.copy) or "dve"
DRAIN = os.environ.get("ATTN_DRAIN", "dve")
# dummy filler matmuls per early qk k-step: keep the PE busy (and the HAM
# clock-gate warm) while phase A's input DMAs trickle in
FILL = int(os.environ.get("ATTN_FILL", "0"))
# normalize multiplies on gpsimd instead of DVE
MUL_GPS = os.environ.get("ATTN_MUL_GPS", "0") == "1"


def _np_mm_dtype():
    return ml_dtypes.bfloat16 if DT_MODE == "bf16" else np.float32


def _mm_dt():
    return mybir.dt.bfloat16 if DT_MODE == "bf16" else mybir.dt.float32


def build():
    nc = bacc.Bacc("TRN2", target_bir_lowering=False, debug=False)
    dt = _mm_dt()
    Exp = mybir.ActivationFunctionType.Exp

    def _mm(ap):
        """View an SBUF AP with the dtype actually fed to the tensor engine."""
        if DT_MODE == "f32r":
            return ap.bitcast(mybir.dt.float32r)
        if ap.dtype == I16:
            return ap.bitcast(mybir.dt.bfloat16)
        return ap

    xT_d = nc.dram_tensor("xT", [D, N], dt, kind="ExternalInput")
    wqkT_d = nc.dram_tensor("wqkT", [D, 2 * D], dt, kind="ExternalInput")
    wvT_d = nc.dram_tensor("wvT", [D, D], dt, kind="ExternalInput")
    wpT_d = nc.dram_tensor("wpT", [D, D], dt, kind="ExternalInput")
    qb_d = nc.dram_tensor("qb", [P, KT], F32, kind="ExternalInput")
    beff_d = nc.dram_tensor("beff", [P, D], F32, kind="ExternalInput")
    out_d = nc.dram_tensor("out", [N, D], dt, kind="ExternalOutput")

    with tile.TileContext(nc) as tc:
        with (
            tc.tile_pool(name="psum", bufs=1, space="PSUM") as psum,
            tc.tile_pool(name="persist", bufs=1) as persist,
            tc.tile_pool(name="work", bufs=1) as work,
        ):
            # ---- PE warm-up: dummy matmuls with no DMA dependency ---------------
            wrm_ps = None
            fill_state = {"n": 0, "ps": None}
            if WARM:
                wrm = work.tile([P, 256], dt, tag="wrm", name="wrm")
                nc.vector.memset(wrm[:], 0.0)
                wrm_ps = psum.tile([P, N], F32, tag="mm2", bufs=2, name="wps")

            def emit_fill(n):
                """Dummy matmuls (one long accumulation into the current fill
                psum) that keep the PE streaming while phase-A DMAs land."""
                if wrm_ps is None or fill_state["ps"] is None:
                    return
                for _ in range(n):
                    nc.tensor.matmul(
                        fill_state["ps"][:, 0:256],
                        _mm(wrm)[:, 0:128], _mm(wrm)[:, 0:256],
                        start=(fill_state["n"] == 0), stop=False,
                    )
                    fill_state["n"] += 1

            def fill_finish(drain_cols):
                """Close the open fill group and give its psum a reader (a
                corner of bb, overwritten by the beff DMA afterwards)."""
                if wrm_ps is None or fill_state["ps"] is None:
                    return
                nc.tensor.matmul(
                    fill_state["ps"][:, 0:256],
                    _mm(wrm)[:, 0:128], _mm(wrm)[:, 0:256],
                    start=False, stop=True,
                )
                nc.vector.tensor_copy(bb[:, drain_cols], fill_state["ps"][:, 0:32])
                fill_state["ps"] = None
                fill_state["n"] = 0

            aprobe = None
            if WARM:
                fill_state["ps"] = wrm_ps
                emit_fill(WARM)
                # tiny dummy exp: pulls the ~2.7us ACT table load (+drain)
                # into the DMA-wait window instead of the first real exp
                aprobe = work.tile([1, 4], F32, tag="aprobe", name="aprobe")
                nc.scalar.activation(aprobe[:], wrm[0:1, 0:4], Exp, scale=1.0)

            # ---- input DMAs (ordered by first use across the 3 DGE queues) -----
            # host permutes w_qkv columns pairwise [q0,k0,q1,k1,...] so the
            # first two q/k pairs' weights (wqkA) are contiguous and land
            # first; x streams on sync, weights on scalar, everything not
            # needed until later (wv/wp/beff) rides the slower gpsimd queue.
            # x is the phase-A pace-setter: split it across the sync (x0-2)
            # and gpsimd (x3-5) queues so tiles land in parallel; wv follows
            # x0-2 on sync, wp/beff (needed only at proj) follow x3-5.
            x_sb, wqkA_sb, wqkB_sb, wv_sb = [], [], [], []
            for k in range(KT):
                xk = persist.tile([P, N], dt, tag=f"x{k}", name=f"x{k}")
                x_sb.append(xk)
            for k in range(3):
                nc.sync.dma_start(x_sb[k][:], xT_d.ap()[k * P : (k + 1) * P, :])
            for k in range(3, KT):
                nc.gpsimd.dma_start(x_sb[k][:], xT_d.ap()[k * P : (k + 1) * P, :])
            qb_sb = persist.tile([P, KT], F32, tag="qb", name="qb")
            nc.scalar.dma_start(qb_sb[:], qb_d.ap())
            for k in range(KT):
                wqka = persist.tile([P, 512], dt, tag=f"wqkA{k}", name=f"wqkA{k}")
                nc.scalar.dma_start(wqka[:], wqkT_d.ap()[k * P : (k + 1) * P, 0:512])
                wqkA_sb.append(wqka)
            for k in range(KT):
                wvk = persist.tile([P, D], dt, tag=f"wv{k}", name=f"wv{k}")
                nc.sync.dma_start(wvk[:], wvT_d.ap()[k * P : (k + 1) * P, :])
                wv_sb.append(wvk)
            for k in range(KT):
                wqkb = persist.tile([P, 1024], dt, tag=f"wqkB{k}", name=f"wqkB{k}")
                nc.scalar.dma_start(
                    wqkb[:], wqkT_d.ap()[k * P : (k + 1) * P, 512 : 2 * D]
                )
                wqkB_sb.append(wqkb)
            wp_sb = []
            for k in range(KT):
                wpk = work.tile([P, D], dt, tag=f"wp{k}", name=f"wp{k}")
                nc.gpsimd.dma_start(wpk[:], wpT_d.ap()[k * P : (k + 1) * P, :])
                wp_sb.append(wpk)
            bb = work.tile([P, D], F32, tag="bb", name="bb")

            def wqk_slice(m, k):
                """lhsT slice for q/k feature tile m from the pair-permuted
                weight layout: q_m at permuted block 2m, k_m at 2(m-KT)+1."""
                pos = 2 * m if m < KT else 2 * (m - KT) + 1
                if pos < 4:
                    return wqkA_sb[k][:, (pos % 4) * P : (pos % 4 + 1) * P]
                return wqkB_sb[k][:, (pos - 4) * P : (pos - 3) * P]

            # ---- emitters ------------------------------------------------------
            v_sb = [None] * TT

            def emit_v(t):
                vt = persist.tile([P, H, HD + 1], dt, tag=f"v{t}", name=f"v{t}")
                nc.vector.memset(vt[:, :, HD], 1.0)
                ps = psum.tile([P, N], F32, tag="mm2", bufs=2, name=f"vps{t}")
                for k in range(KT):
                    lhsT = _mm(x_sb[k])[:, t * P : (t + 1) * P]
                    nc.tensor.matmul(
                        ps[:, 0:512], lhsT, _mm(wv_sb[k])[:, 0:512],
                        start=(k == 0), stop=(k == KT - 1),
                    )
                    nc.tensor.matmul(
                        ps[:, 512:768], lhsT, _mm(wv_sb[k])[:, 512:768],
                        start=(k == 0), stop=(k == KT - 1),
                    )
                nc.vector.tensor_copy(
                    vt[:, :, 0:HD], ps[:, 0:768].rearrange("p (h d) -> p h d", d=HD)
                )
                v_sb[t] = vt

            qk_sb = [None] * MQ

            def open_qk(m, tag="mm2"):
                """Allocate the psum for q/k tile m; matmuls stream in via
                qk_step so they can be slotted between other work."""
                return psum.tile([P, N], F32, tag=tag, bufs=2, name=f"qkps{m}")

            def qk_step(m, ps, k):
                lhsT = _mm(wqk_slice(m, k))
                for half in range(2):
                    nc.tensor.matmul(
                        ps[:, half * 512 : (half + 1) * 512],
                        lhsT,
                        _mm(x_sb[k])[:, half * 512 : (half + 1) * 512],
                        start=(k == 0), stop=(k == KT - 1),
                    )

            def close_qk(m, ps):
                qkm = persist.tile([P, N], dt, tag=f"qk{m}", name=f"qk{m}")
                if m < KT:
                    # q tile: add q_bias (per-partition scalar in feature-major);
                    # on DVE to keep ACT free for the softmax exps
                    nc.vector.tensor_scalar_add(qkm[:], ps[:], qb_sb[:, m : m + 1])
                else:
                    nc.vector.tensor_copy(qkm[:], ps[:])
                qk_sb[m] = qkm

            def emit_qk(m, fills=0):
                ps = open_qk(m)
                for k in range(KT):
                    qk_step(m, ps, k)
                    emit_fill(fills)
                close_qk(m, ps)

            attn_sb = [
                persist.tile([P, N], dt, tag=f"attn{g}", name=f"attn{g}")
                for g in range(KT)
            ]
            pt_tiles = {}  # (g, j, half) -> exp'd S^T tile, consumed by emit_pv
            pt_reads = {}

            def emit_sx(g, j):
                """S^T matmuls + exp for pair g, key tile j (both heads).

                Layout trick: per query-half, BOTH heads' S go into ONE psum
                tile — h0 at cols 0:512, h1 at cols 512:1024.  The two K=64
                matmuls occupy disjoint PE row groups (tile_position) AND
                disjoint PSUM banks, so they issue back-to-back and run fully
                concurrent (HW-measured 110 ns/MM vs 216 serial).  One exp
                instruction then covers both heads at no extra ACT cost.
                """
                q_t, k_t = qk_sb[g], qk_sb[KT + g]
                for half in range(2):
                    qcols = slice(half * 512, (half + 1) * 512)
                    sp = psum.tile(
                        [P, N], F32, tag="sp", bufs=2, name=f"sp{g}_{j}_{half}"
                    )
                    for hh in range(2):
                        lhsT = _mm(k_t)[hh * HD : (hh + 1) * HD, j * P : (j + 1) * P]
                        rhs = _mm(q_t)[hh * HD : (hh + 1) * HD, qcols]
                        nc.tensor.matmul(
                            sp[:, hh * 512 : (hh + 1) * 512], lhsT, rhs,
                            tile_position=(hh * HD, 0),
                        )
                    if j in SCHRAUD_JS:
                        # Schraudolph exp on DVE: one tensor_scalar; the int16
                        # result IS the bf16 bit pattern of ~exp(x)
                        pt = work.tile(
                            [P, N], I16, tag="pt1", bufs=12, name=f"pt{g}_{j}_{half}"
                        )
                        nc.vector.tensor_scalar(
                            pt[:], sp[:], A16 * SCALE, B16,
                            op0=mybir.AluOpType.mult, op1=mybir.AluOpType.add,
                        )
                    else:
                        pt = work.tile(
                            [P, N], dt, tag="pt", bufs=24, name=f"pt{g}_{j}_{half}"
                        )
                        nc.scalar.activation(pt[:], sp[:], Exp, scale=SCALE)
                    pt_tiles[(g, j, half)] = pt

            def open_pv(g, hh):
                return psum.tile(
                    [HD + 1, N], F32, tag="mm2", bufs=2, name=f"pv{g}_{hh}"
                )

            def pv_step(g, hh, pp, j):
                # pt tiles hold [h0 | h1] per query-half; head hh's probs for
                # query-half q live at ptq[:, hh*512:(hh+1)*512]
                lhsT = _mm(v_sb[j])[:, 2 * g + hh, :]  # [128, 65] V|1
                for half in range(2):
                    # each half-tile is read by both heads; free after 2nd read
                    key = (g, j, half)
                    pt = pt_tiles[key]
                    pt_reads[key] = pt_reads.get(key, 0) + 1
                    if pt_reads[key] == 2:
                        del pt_tiles[key]
                    nc.tensor.matmul(
                        pp[:, half * 512 : (half + 1) * 512],
                        lhsT,
                        _mm(pt)[:, hh * 512 : (hh + 1) * 512],
                        start=(j == 0), stop=(j == TT - 1),
                    )

            def close_pv(g, hh, pp):
                # drain the PSUM accumulator right away so the 2-slot "mm2"
                # rotation unblocks the interleaved q/k matmuls
                sb = work.tile(
                    [HD + 1, N], F32, tag="ppsb", bufs=2, name=f"ppsb{g}{hh}"
                )
                if DRAIN == "act":
                    nc.scalar.copy(sb[:], pp[:])
                else:
                    nc.vector.tensor_copy(sb[:], pp[:])
                # normalize: row HD of sb is the softmax denominator
                rrow0 = work.tile([1, N], F32, tag="rr0", bufs=2, name=f"rr0{g}{hh}")
                rb = work.tile([HD, N], F32, tag="rb", bufs=2, name=f"rb{g}{hh}")
                # s hops to partition 0 (DMA moves across lanes)...
                nc.sync.dma_start(rrow0[:], sb[HD : HD + 1, :])
                # ...1/s at base 0 (approx_fast is ~51 ULP, far below the
                # bf16 noise floor, 5x cheaper than InstReciprocal, and
                # only correct on base-partition-0 APs)...
                nc.vector.reciprocal_approx_fast(rrow0[:], rrow0[:])
                # ...and fan out to all 64 lanes (gpsimd reads partition 0)
                nc.gpsimd.partition_broadcast(rb[:], rrow0[:])
                if hh == 0:
                    nc.vector.tensor_mul(attn_sb[g][0:HD, :], sb[0:HD, :], rb[:])
                else:
                    # PV output lives at partitions 0..64 but this head's
                    # slot in attn_sb is partitions 64..127; DVE lanes
                    # can't shift partitions, so normalize at base 0 and
                    # bounce across partitions with an SBUF->SBUF DMA
                    # (split across both hwdge queues: this bounce sits on
                    # the epilogue critical path before proj's k=5).
                    stg = work.tile([HD, N], dt, tag="stg", bufs=2, name=f"stg{g}")
                    nc.vector.tensor_mul(stg[:], sb[0:HD, :], rb[:])
                    nc.sync.dma_start(attn_sb[g][HD:P, 0:512], stg[:, 0:512])
                    nc.scalar.dma_start(attn_sb[g][HD:P, 512:N], stg[:, 512:N])

            # ---- emission order: software pipeline -----------------------------
            # Tile's per-engine instruction order is (near-)emission order and
            # a stalled instruction head-of-line blocks its engine, so filler
            # work is emitted BEFORE each exp-paced S step. Steady state: pair
            # g's slots carry pair g-1's PV (heads serialized so only one PV
            # accumulator is live -> one free PSUM slot) plus pair g+1's q/k
            # matmuls, so the exp stream never waits on a PE-only block.
            # all four early q/k pairs (they all read the early wqkA block)
            # stream k-major so each x-tile arrival feeds 8 matmuls: pairs 0/6
            # in the mm2 slots, pairs 1/7 borrow the (still unused) sp slots
            fill_finish(slice(0, 32))
            if aprobe is not None:
                # give the probe a reader (bb corner, overwritten by beff DMA)
                nc.vector.tensor_copy(bb[0:1, 60:64], aprobe[:])
            qk_ps = {
                0: open_qk(0), KT: open_qk(KT),
                1: open_qk(1, "sp"), KT + 1: open_qk(KT + 1, "sp"),
            }
            for k in range(KT):
                for m in (0, KT, 1, KT + 1):
                    qk_step(m, qk_ps[m], k)
                emit_fill(FILL)
            for m in (0, KT, 1, KT + 1):
                close_qk(m, qk_ps[m])
            nc.gpsimd.dma_start(bb[:], beff_d.ap())
            for j in range(TT):
                emit_sx(0, j)
                emit_v(j)
            pp51 = None
            for g in range(1, KT):
                nxt = g + 1 if g + 1 < KT else None
                pp = qkps = m = None
                for slot in range(TT):
                    emit_sx(g, slot)
                    hh = 0 if slot < 4 else 1
                    if slot in (0, 4):
                        pp = open_pv(g - 1, hh)
                        if nxt is not None:
                            m = nxt if slot == 0 else KT + nxt
                            qkps = open_qk(m)
                    pv_step(g - 1, hh, pp, 2 * (slot % 4))
                    pv_step(g - 1, hh, pp, 2 * (slot % 4) + 1)
                    if nxt is not None:
                        # finish the k-steps by slot 2/6 so the DVE close cast
                        # lands well before the next iteration's S matmuls
                        # need the q/k tile (this cast was the ~0.9us
                        # iteration-boundary PE stall)
                        for k in ([0, 1, 2], [3, 4], [5], [])[slot % 4]:
                            qk_step(m, qkps, k)
                        if slot in (2, 6):
                            close_qk(m, qkps)
                    if slot in (3, 7):
                        close_pv(g - 1, hh, pp)
            # ---- phase C: out = attn @ w_proj^T + beff -------------------------
            def proj_steps(t, ps, ks):
                for k in ks:
                    lhsT = _mm(attn_sb[k])[:, t * P : (t + 1) * P]
                    nc.tensor.matmul(
                        ps[:, 0:512], lhsT, _mm(wp_sb[k])[:, 0:512],
                        start=(k == 0), stop=(k == KT - 1),
                    )
                    nc.tensor.matmul(
                        ps[:, 512:768], lhsT, _mm(wp_sb[k])[:, 512:768],
                        start=(k == 0), stop=(k == KT - 1),
                    )

            def proj_close(t, ps):
                ot = work.tile([P, D], dt, tag="ot", bufs=3, name=f"ot{t}")
                nc.vector.tensor_add(ot[:], ps[:, 0:768], bb[:])
                # alternate the output stores over the two hwdge queues so
                # the 1.5MB of stores don't serialize into a tail; the very
                # last store splits across both queues to halve the tail
                if t == TT - 1:
                    nc.sync.dma_start(out_d.ap()[t * P : t * P + HD, :], ot[0:HD, :])
                    nc.scalar.dma_start(out_d.ap()[t * P + HD : (t + 1) * P, :], ot[HD:P, :])
                else:
                    eng = (nc.sync, nc.scalar)[t % 2]
                    eng.dma_start(out_d.ap()[t * P : (t + 1) * P, :], ot[:])

            # epilogue: the last pair's PV + normalize interleaved with proj
            # k=0..4 partials (which need only attn[0..4]) so the PE never
            # idles on the exp stream or the normalize chain; proj psums
            # alternate between the two pools -> 4 accumulators in flight.
            def proj_open(t):
                tag = "sp" if t in (0, 1, 4, 6) else "mm2"
                return psum.tile([P, N], F32, tag=tag, bufs=2, name=f"ops{t}")

            # head 1 first: its normalize chain is longer (partition-bounce
            # DMA), so it hides under head 0's PV and the proj partials
            proj_ps = {}
            pp51 = open_pv(KT - 1, 1)
            for j in range(TT):
                pv_step(KT - 1, 1, pp51, j)
            close_pv(KT - 1, 1, pp51)
            pp0 = open_pv(KT - 1, 0)
            for j in range(TT):
                pv_step(KT - 1, 0, pp0, j)
            close_pv(KT - 1, 0, pp0)
            # all k=0..4 partials AFTER both closes: ~8us of ready PE work
            # covering the last normalize chains before anything needs attn[5]
            for t in range(4):
                proj_ps[t] = proj_open(t)
                proj_steps(t, proj_ps[t], range(KT - 1))
            for t in range(4):
                proj_steps(t, proj_ps[t], [KT - 1])
                proj_close(t, proj_ps[t])
            for t in range(4, TT):
                ps = proj_open(t)
                proj_steps(t, ps, range(KT))
                proj_close(t, ps)

    nc.compile()
    return nc


_NC_CACHE = None


def _get_nc():
    global _NC_CACHE
    if _NC_CACHE is None:
        _NC_CACHE = build()
    return _NC_CACHE


def make_in_maps(x, w_qkv, q_bias, v_bias, w_proj, b_proj):
    mmdt = _np_mm_dtype()
    wqkT = np.ascontiguousarray(w_qkv[: 2 * D].T)
    # pair-permute the q/k feature blocks: [q0,k0,q1,k1,...] so the first two
    # pairs' weights are one contiguous early DMA (see wqk_slice in build)
    perm = []
    for m in range(KT):
        perm.extend(range(m * P, (m + 1) * P))
        perm.extend(range((KT + m) * P, (KT + m + 1) * P))
    wqkT = np.ascontiguousarray(wqkT[:, perm]).astype(mmdt)
    wvT = np.ascontiguousarray(w_qkv[2 * D :].T).astype(mmdt)
    wpT = np.ascontiguousarray(w_proj.T).astype(mmdt)
    qb = np.ascontiguousarray(q_bias.reshape(KT, P).T).astype(np.float32)
    beff_row = (
        w_proj.astype(np.float64) @ v_bias.astype(np.float64) + b_proj
    ).astype(np.float32)
    beff = np.ascontiguousarray(np.tile(beff_row, (P, 1)))
    shared = {"wqkT": wqkT, "wvT": wvT, "wpT": wpT, "qb": qb, "beff": beff}
    in_maps = []
    for i in range(N_CORES):
        m = dict(shared)
        m["xT"] = np.ascontiguousarray(x[i].T).astype(mmdt)
        in_maps.append(m)
    return in_maps


def kernel(x, w_qkv, q_bias, v_bias, w_proj, b_proj, _trace=False, _tmpdir=None):
    x = np.asarray(x)
    nc = _get_nc()
    in_maps = make_in_maps(
        np.asarray(x, dtype=np.float32),
        np.asarray(w_qkv, dtype=np.float32),
        np.asarray(q_bias, dtype=np.float32),
        np.asarray(v_bias, dtype=np.float32),
        np.asarray(w_proj, dtype=np.float32),
        np.asarray(b_proj, dtype=np.float32),
    )
    res = run_bass_kernel_spmd(
        nc, in_maps, core_ids=list(range(N_CORES)), trace=_trace, tmpdir=_tmpdir
    )
    out = np.stack(
        [np.asarray(res.results[i]["out"]).astype(np.float32) for i in range(N_CORES)],
        axis=0,
    )
    if _trace:
        return out, res
    return out
